# revision 16
# baseline (speedup 1.0000x reference)
"""Multi-head causal attention (B=8, T=2048, C=128, H=4, Dh=32) on 8 trn2
NeuronCores — one batch element per core, fully data-parallel.

v2 design (everything laid out transposed; no on-device transposes):
  - qT/kT = W.T @ x.T               [C, T] f32r (head h = partitions 32h..)
  - V natural [s, (h d)] bf16 per 128-row s-block
  - scores computed transposed per (s-block j, t-chunk c, head-pair):
    S^T = kT_h.T @ qT_h via row-tiled matmuls (tile_position=(32h,0)) -> two
    heads run concurrently in the PE array; out [128, 2x512] PSUM
  - softmax exp split across THREE engines: ScalarE runs exact Exp;
    VectorE/GpSimd run a Schraudolph bit-trick exp (int16(A*x+B) bitcast to
    bf16, ~1.8% rms) on a weighted round-robin schedule. P stored bf16.
  - causal diag blocks: exp computed on cols >= 128m only; the 128x128
    triangle masked by one strided DVE multiply with a 4x-tiled tri matrix
  - PV via 4x col-tiled matmuls (tile_position=(0,32h)): all 4 heads
    concurrently, out O^T accumulated [32 rows/head, 512] in one PSUM bank;
    denominators via a second col-tiled span with a ones-column stationary
    (M=1) accumulating into another bank at partitions {0,32,64,96}
  - normalization: sums rows copied to SBUF, broadcast to [128,512] via a
    head-selector matmul, reciprocal + multiply on DVE, then out projection
  - final^T = Wo.T @ (O^T * R) -> [C, T] f32; host transposes back.
"""
import sys

sys.path.insert(0, "/opt/trn_rl_repo")
import numpy as np

B, T, C, H, Dh = 8, 2048, 128, 4, 32
CW = 512                 # t-chunk width
NCH = T // CW            # 4 chunks
NBLK = T // 128          # 16 s-blocks
NPAIR = 2 * sum(4 * (c + 1) for c in range(NCH))   # 80 (j,c,pair) exp instrs
SCALE = float(1.0 / np.sqrt(np.float32(Dh)))
A16 = float(128.0 / np.log(2.0)) * SCALE   # Schraudolph slope (bf16 target)
B16 = 16249.5                              # 127*128 - C, C~6.5/7.5
EXPMIX = (42, 0, 38)                       # ACT / Pool / DVE exp shares
PRING = 5                                  # PBUF ring depth
PVLAG = 4                                  # software-pipeline lag for PV/sums


def exp_engine_pattern(weights, n):
    """Deterministic weighted round-robin over engines A/P/D."""
    keys = ("A", "P", "D")
    total = float(sum(weights))
    acc = [0.0, 0.0, 0.0]
    out = []
    for _ in range(n):
        for i in range(3):
            acc[i] += weights[i] / total
        i = max(range(3), key=lambda k: acc[k])
        acc[i] -= 1.0
        out.append(keys[i])
    return out


def _split_excess_waits(nc):
    """This walrus build accepts only ONE semaphore wait per engine
    instruction. Move excess waits onto injected same-engine NoOps."""
    import bass_rust
    from concourse import mybir

    for f in nc.m.functions:
        for blk in f.blocks:
            out = []
            for ins in blk.instructions:
                si = getattr(ins, "sync_info", None)
                if si is not None:
                    waits = list(si.on_wait)
                    movable = [w for w in waits if w.wait_reg is None]
                    if len(waits) > 1 and len(movable) == len(waits):
                        for k, w in enumerate(waits):
                            nop = mybir.InstNoOp(
                                name=f"{ins.name}-wsplit{k}", ins=[], outs=[]
                            )
                            nop.engine = ins.engine
                            nop.sync_info = bass_rust.SyncInfo(
                                on_wait=[w], on_update=[]
                            )
                            out.append(nop)
                        ins.sync_info = bass_rust.SyncInfo(
                            on_wait=[], on_update=list(si.on_update)
                        )
                out.append(ins)
            blk.instructions = out
    return nc


def build_nc(mm="f32r", iters=1, bias_qk=False, bias_v=False, bias_o=False,
             fixup=True, phase="full", expmix=EXPMIX):
    import contextlib

    import concourse.bass as bass
    import concourse.tile as tile
    from concourse import mybir

    f32 = mybir.dt.float32
    bf16 = mybir.dt.bfloat16
    i16 = mybir.dt.int16
    dtm = bf16   # whole matmul pipeline in bf16: FWL + PE tile concurrency
    EXP = mybir.ActivationFunctionType.Exp
    MUL = mybir.AluOpType.mult
    ADD = mybir.AluOpType.add

    nc = bass.Bass()
    xT_d = nc.dram_tensor("xT", [C, T], dtm, kind="ExternalInput")
    Wq_d = nc.dram_tensor("Wq", [C, C], dtm, kind="ExternalInput")
    Wk_d = nc.dram_tensor("Wk", [C, C], dtm, kind="ExternalInput")
    Wv_d = nc.dram_tensor("Wv", [C, C], dtm, kind="ExternalInput")
    Wo_d = nc.dram_tensor("Wo", [C, C], dtm, kind="ExternalInput")
    sel_d = nc.dram_tensor("sel", [C, C], dtm, kind="ExternalInput")
    tri4_d = nc.dram_tensor("tri4", [128, 4 * 128], bf16, kind="ExternalInput")
    if bias_qk:
        bq_d = nc.dram_tensor("bq", [C, 1], f32, kind="ExternalInput")
        bk_d = nc.dram_tensor("bk", [C, 1], f32, kind="ExternalInput")
    if bias_v:
        bvb_d = nc.dram_tensor("bvb", [128, C], f32, kind="ExternalInput")
    if bias_o:
        bo_d = nc.dram_tensor("bo", [C, 1], f32, kind="ExternalInput")
    outT_d = nc.dram_tensor("outT", [C, T], f32, kind="ExternalOutput")

    sched = exp_engine_pattern(expmix, NPAIR)

    with tile.TileContext(nc) as tc:
        with contextlib.ExitStack() as ctx:
            ctx.enter_context(
                nc.allow_low_precision(reason="mixed precision attention")
            )
            consts = ctx.enter_context(tc.tile_pool(name="consts", bufs=1))
            work = ctx.enter_context(tc.tile_pool(name="work", bufs=2))
            wk2 = ctx.enter_context(tc.tile_pool(name="wk2", bufs=2))
            psS = ctx.enter_context(tc.tile_pool(name="psS", bufs=3, space="PSUM"))
            psM = ctx.enter_context(tc.tile_pool(name="psM", bufs=1, space="PSUM"))

            # persistent PSUM accumulators: the sums bank's non-head rows are
            # zeroed once here and never touched again, so a plain full-bank
            # copy (engines cannot stride the partition dim) reads zeros there
            pvacc = psM.tile([128, CW], f32, tag="pv")
            sumacc = psM.tile([128, CW], f32, tag="sm")
            nc.vector.memset(sumacc[:], 0.0)

            # ---- constants (outside the timing loop) ----
            xT = consts.tile([C, T], dtm)
            Wq = consts.tile([C, C], dtm)
            Wk = consts.tile([C, C], dtm)
            Wv = consts.tile([C, C], dtm)
            Wo = consts.tile([C, C], dtm)
            sel = consts.tile([C, C], dtm)
            tri4 = consts.tile([128, 4 * 128], bf16)
            nc.sync.dma_start(out=xT[:], in_=xT_d[:])
            nc.sync.dma_start(out=Wq[:], in_=Wq_d[:])
            nc.sync.dma_start(out=Wk[:], in_=Wk_d[:])
            nc.sync.dma_start(out=Wv[:], in_=Wv_d[:])
            nc.sync.dma_start(out=Wo[:], in_=Wo_d[:])
            nc.sync.dma_start(out=sel[:], in_=sel_d[:])
            nc.sync.dma_start(out=tri4[:], in_=tri4_d[:])
            if bias_qk:
                bq = consts.tile([C, 1], f32)
                bk = consts.tile([C, 1], f32)
                nc.sync.dma_start(out=bq[:], in_=bq_d[:])
                nc.sync.dma_start(out=bk[:], in_=bk_d[:])
            if bias_v:
                bvb = consts.tile([128, C], f32)
                nc.sync.dma_start(out=bvb[:], in_=bvb_d[:])
            if bias_o:
                bo = consts.tile([C, 1], f32)
                nc.sync.dma_start(out=bo[:], in_=bo_d[:])
            onescol = consts.tile([128, 1], bf16)
            # bf16 memset is rejected by codegen; fill via x*0 + 1
            nc.vector.tensor_scalar(onescol[:], tri4[:, 0:1], 0.0, 1.0,
                                    MUL, ADD)
            # sums landing pad: only rows {0,32,64,96} are ever written, the
            # rest must be zero for the selector matmul
            S1sb = consts.tile([128, NCH * CW], dtm)
            nc.vector.memset(S1sb[:].bitcast(mybir.dt.uint32), 0)

            tri4v = tri4[:].rearrange("p (h q) -> p h q", q=128)

            def body():
                qT = work.tile([C, T], dtm, tag="qT")
                kT = work.tile([C, T], dtm, tag="kT")
                v1 = work.tile([128, NBLK * H * Dh], bf16, tag="v1")
                v1v = v1.rearrange("p (j h e) -> p j h e", h=H, e=Dh)
                PBUF = work.tile([128, PRING * H * CW], bf16, tag="PBUF")

                units = [(c, j) for c in range(NCH)
                         for j in range(4 * (c + 1))]

                def emit_qkv(c):
                    cs = slice(c * CW, (c + 1) * CW)
                    qk = psS.tile([128, 1024], f32, tag="ps")
                    nc.tensor.matmul(qk[:, 0:CW], Wk[:], xT[:, cs],
                                     start=True, stop=True)
                    nc.tensor.matmul(qk[:, CW:2 * CW], Wq[:], xT[:, cs],
                                     start=True, stop=True)
                    if bias_qk:
                        nc.scalar.add(kT[:, cs], qk[:, 0:CW], bk)
                        nc.scalar.add(qT[:, cs], qk[:, CW:2 * CW], bq)
                    else:
                        nc.scalar.copy(kT[:, cs], qk[:, 0:CW])
                        nc.scalar.copy(qT[:, cs], qk[:, CW:2 * CW])
                    vs = psS.tile([128, 1024], f32, tag="ps")
                    for mj in range(4):
                        j = 4 * c + mj
                        nc.tensor.matmul(
                            vs[:, 128 * mj:128 * (mj + 1)],
                            xT[:, j * 128:(j + 1) * 128], Wv[:],
                            start=True, stop=True,
                        )
                    vsv = vs[:, 0:512].rearrange("p (j h e) -> p j h e",
                                                 h=H, e=Dh)
                    if bias_v:
                        bvbv = bvb[:].rearrange("p (h e) -> p h e", e=Dh)
                        for mj in range(4):
                            nc.vector.tensor_add(
                                v1v[:, 4 * c + mj, :, :], vsv[:, mj, :, :],
                                bvbv,
                            )
                    else:
                        nc.scalar.copy(v1v[:, 4 * c:4 * c + 4, :, :], vsv)

                def emit_scores_exp(idx, c, j):
                    m = j - 4 * c
                    lo = 128 * m if m > 0 else 0
                    P = PBUF[:, (idx % PRING) * H * CW:
                             ((idx % PRING) + 1) * H * CW]
                    Pv4 = P.rearrange("p (h q) -> p h q", q=CW)
                    for pi, (he, ho) in enumerate(((0, 1), (2, 3))):
                        sc = psS.tile([128, 1024], f32, tag="ps")
                        for col, h in ((0, he), (CW, ho)):
                            nc.tensor.matmul(
                                sc[:, col + lo:col + CW],
                                kT[32 * h:32 * h + 32,
                                   j * 128:(j + 1) * 128],
                                qT[32 * h:32 * h + 32,
                                   c * CW + lo:(c + 1) * CW],
                                start=True, stop=True,
                                tile_position=(32 * h, 0),
                            )
                        eng = sched[2 * idx + pi]
                        if lo:
                            pin = sc[:].rearrange(
                                "p (g q) -> p g q", q=CW)[:, :, lo:CW]
                            pout = Pv4[:, 2 * pi:2 * pi + 2, lo:CW]
                        else:
                            pin = sc[:]
                            pout = P[:, 1024 * pi:1024 * (pi + 1)]
                        if eng == "A":
                            nc.scalar.activation(pout, pin, EXP, scale=SCALE)
                        else:
                            nc.vector.tensor_scalar(
                                pout.bitcast(i16), pin, A16, B16, MUL, ADD)
                    if m >= 0:  # diagonal block: causal triangle mask
                        mv = Pv4[:, :, 128 * m:128 * (m + 1)]
                        nc.gpsimd.tensor_mul(mv, mv, tri4v)

                def emit_pv(idx, c, j):
                    m = j - 4 * c
                    lo = 128 * m if m > 0 else 0
                    nblk = 4 * (c + 1)
                    P = PBUF[:, (idx % PRING) * H * CW:
                             ((idx % PRING) + 1) * H * CW]
                    Pv4 = P.rearrange("p (h q) -> p h q", q=CW)
                    for h in range(H):
                        nc.tensor.matmul(
                            pvacc[32 * h:32 * h + 32, lo:CW],
                            v1v[:, j, h, :],
                            Pv4[:, h, lo:CW],
                            start=(j == 0), stop=(j == nblk - 1),
                            tile_position=(0, 32 * h),
                            skip_group_check=True,
                        )
                    for h in range(H):
                        nc.tensor.matmul(
                            sumacc[32 * h:32 * h + 1, lo:CW],
                            onescol[:],
                            Pv4[:, h, lo:CW],
                            start=(j == 0), stop=(j == nblk - 1),
                            tile_position=(0, 32 * h),
                            skip_group_check=True,
                        )

                def emit_chunk_end(c):
                    # reuse the freed accumulator banks: R broadcast lands in
                    # sumacc (after S1 copy), the projection in pvacc (after
                    # the OTs read) — no scores slot is stolen at chunk end
                    cs = slice(c * CW, (c + 1) * CW)
                    S1c = S1sb[:, c * CW:(c + 1) * CW]
                    nc.vector.tensor_copy(S1c, sumacc[:])
                    nc.tensor.matmul(sumacc[:], sel[:], S1c,
                                     start=True, stop=True)
                    Rsb = wk2.tile([128, CW], f32, tag="R", name=f"R{c}")
                    nc.vector.reciprocal(Rsb[:], sumacc[:])
                    OTs = wk2.tile([128, CW], dtm, tag="OTs", name=f"OTs{c}")
                    nc.vector.tensor_mul(OTs[:], pvacc[:], Rsb[:])
                    nc.tensor.matmul(pvacc[:], Wo[:], OTs[:],
                                     start=True, stop=True)
                    outsb = wk2.tile([128, CW], f32, tag="ob", name=f"ob{c}")
                    if bias_o:
                        nc.scalar.add(outsb[:], pvacc[:], bo)
                    else:
                        nc.scalar.copy(outsb[:], pvacc[:])
                    nc.sync.dma_start(out=outT_d[:, cs], in_=outsb[:])

                def emit_pv_and_end(idx):
                    c, j = units[idx]
                    emit_pv(idx, c, j)
                    if j == 4 * (c + 1) - 1:   # last s-block of chunk c
                        emit_chunk_end(c)

                for idx, (c, j) in enumerate(units):
                    if j == 0:
                        emit_qkv(c)
                    emit_scores_exp(idx, c, j)
                    if idx >= PVLAG:
                        emit_pv_and_end(idx - PVLAG)
                for idx in range(len(units) - PVLAG, len(units)):
                    emit_pv_and_end(idx)

            if iters == 1:
                body()
            else:
                from concourse import mybir as _mb

                with tc.For_i(0, iters, 1, hint_engines=(_mb.EngineType.PE,)):
                    body()

    return _split_excess_waits(nc) if fixup else nc


def _host_inputs(x, W_qkv, b_qkv, W_out, b_out):
    import ml_dtypes

    f = np.float32
    b16 = ml_dtypes.bfloat16
    Wq = np.ascontiguousarray(W_qkv[:, 0:C]).astype(b16)
    Wk = np.ascontiguousarray(W_qkv[:, C:2 * C]).astype(b16)
    Wv = np.ascontiguousarray(W_qkv[:, 2 * C:3 * C]).astype(b16)
    Wo = np.ascontiguousarray(W_out).astype(b16)
    sel = np.zeros((C, C), f)
    for g in range(H):
        sel[32 * g, 32 * g:32 * g + 32] = 1.0
    sel = sel.astype(b16)
    tri4 = np.tile(np.triu(np.ones((128, 128), f)), (1, 4)).astype(b16)
    bias_qk = bool(np.any(b_qkv[0:2 * C]))
    bias_v = bool(np.any(b_qkv[2 * C:3 * C]))
    bias_o = bool(np.any(b_out))
    base = {"Wq": Wq, "Wk": Wk, "Wv": Wv, "Wo": Wo, "sel": sel, "tri4": tri4}
    if bias_qk:
        base["bq"] = np.ascontiguousarray(b_qkv[0:C].reshape(C, 1), f)
        base["bk"] = np.ascontiguousarray(b_qkv[C:2 * C].reshape(C, 1), f)
    if bias_v:
        base["bvb"] = np.tile(b_qkv[2 * C:3 * C].reshape(1, C),
                              (128, 1)).astype(f)
    if bias_o:
        base["bo"] = np.ascontiguousarray(b_out.reshape(C, 1), f)
    in_maps = []
    for b in range(B):
        m = dict(base)
        m["xT"] = np.ascontiguousarray(x[b].T).astype(b16)
        in_maps.append(m)
    return in_maps, bias_qk, bias_v, bias_o


def kernel(x, W_qkv, b_qkv, W_out, b_out):
    from concourse.bass_utils import run_bass_kernel_spmd

    in_maps, bias_qk, bias_v, bias_o = _host_inputs(x, W_qkv, b_qkv, W_out,
                                                    b_out)
    nc = build_nc(mm="f32r", iters=1,
                  bias_qk=bias_qk, bias_v=bias_v, bias_o=bias_o)
    res = run_bass_kernel_spmd(nc, in_maps, core_ids=list(range(B)))
    out = np.stack([res.results[b]["outT"].T for b in range(B)])
    return np.ascontiguousarray(out, np.float32)


# revision 17
# speedup vs baseline: 1.1332x; 1.1332x over previous
"""Multi-head causal attention (B=8, T=2048, C=128, H=4, Dh=32) on 8 trn2
NeuronCores — one batch element per core, fully data-parallel.

v2 design (everything laid out transposed; no on-device transposes):
  - qT/kT = W.T @ x.T               [C, T] f32r (head h = partitions 32h..)
  - V natural [s, (h d)] bf16 per 128-row s-block
  - scores computed transposed per (s-block j, t-chunk c, head-pair):
    S^T = kT_h.T @ qT_h via row-tiled matmuls (tile_position=(32h,0)) -> two
    heads run concurrently in the PE array; out [128, 2x512] PSUM
  - softmax exp split across THREE engines: ScalarE runs exact Exp;
    VectorE/GpSimd run a Schraudolph bit-trick exp (int16(A*x+B) bitcast to
    bf16, ~1.8% rms) on a weighted round-robin schedule. P stored bf16.
  - causal diag blocks: exp computed on cols >= 128m only; the 128x128
    triangle masked by one strided DVE multiply with a 4x-tiled tri matrix
  - PV via 4x col-tiled matmuls (tile_position=(0,32h)): all 4 heads
    concurrently, out O^T accumulated [32 rows/head, 512] in one PSUM bank;
    denominators via a second col-tiled span with a ones-column stationary
    (M=1) accumulating into another bank at partitions {0,32,64,96}
  - normalization: sums rows copied to SBUF, broadcast to [128,512] via a
    head-selector matmul, reciprocal + multiply on DVE, then out projection
  - final^T = Wo.T @ (O^T * R) -> [C, T] f32; host transposes back.
"""
import sys

sys.path.insert(0, "/opt/trn_rl_repo")
import numpy as np

B, T, C, H, Dh = 8, 2048, 128, 4, 32
CW = 512                 # t-chunk width
NCH = T // CW            # 4 chunks
NBLK = T // 128          # 16 s-blocks
NPAIR = 2 * sum(4 * (c + 1) for c in range(NCH))   # 80 (j,c,pair) exp instrs
SCALE = float(1.0 / np.sqrt(np.float32(Dh)))
A16 = float(128.0 / np.log(2.0)) * SCALE   # Schraudolph slope (bf16 target)
B16 = 16249.5                              # 127*128 - C, C~6.5/7.5
EXPMIX = (42, 0, 38)                       # ACT / Pool / DVE exp shares
PRING = 5                                  # PBUF ring depth
PVLAG = 4                                  # software-pipeline lag for PV/sums


def exp_engine_pattern(weights, n):
    """Deterministic weighted round-robin over engines A/P/D."""
    keys = ("A", "P", "D")
    total = float(sum(weights))
    acc = [0.0, 0.0, 0.0]
    out = []
    for _ in range(n):
        for i in range(3):
            acc[i] += weights[i] / total
        i = max(range(3), key=lambda k: acc[k])
        acc[i] -= 1.0
        out.append(keys[i])
    return out


def _split_excess_waits(nc):
    """This walrus build accepts only ONE semaphore wait per engine
    instruction. Move excess waits onto injected same-engine NoOps."""
    import bass_rust
    from concourse import mybir

    for f in nc.m.functions:
        for blk in f.blocks:
            out = []
            for ins in blk.instructions:
                si = getattr(ins, "sync_info", None)
                if si is not None:
                    waits = list(si.on_wait)
                    movable = [w for w in waits if w.wait_reg is None]
                    if len(waits) > 1 and len(movable) == len(waits):
                        for k, w in enumerate(waits):
                            nop = mybir.InstNoOp(
                                name=f"{ins.name}-wsplit{k}", ins=[], outs=[]
                            )
                            nop.engine = ins.engine
                            nop.sync_info = bass_rust.SyncInfo(
                                on_wait=[w], on_update=[]
                            )
                            out.append(nop)
                        ins.sync_info = bass_rust.SyncInfo(
                            on_wait=[], on_update=list(si.on_update)
                        )
                out.append(ins)
            blk.instructions = out
    return nc


def build_nc(mm="f32r", iters=1, bias_qk=False, bias_v=False, bias_o=False,
             fixup=True, phase="full", expmix=EXPMIX):
    import contextlib

    import concourse.bass as bass
    import concourse.tile as tile
    from concourse import mybir

    f32 = mybir.dt.float32
    bf16 = mybir.dt.bfloat16
    i16 = mybir.dt.int16
    dtm = bf16   # whole matmul pipeline in bf16: FWL + PE tile concurrency
    EXP = mybir.ActivationFunctionType.Exp
    MUL = mybir.AluOpType.mult
    ADD = mybir.AluOpType.add

    nc = bass.Bass()
    xT_d = nc.dram_tensor("xT", [C, T], dtm, kind="ExternalInput")
    Wq_d = nc.dram_tensor("Wq", [C, C], dtm, kind="ExternalInput")
    Wk_d = nc.dram_tensor("Wk", [C, C], dtm, kind="ExternalInput")
    Wv_d = nc.dram_tensor("Wv", [C, C], dtm, kind="ExternalInput")
    Wo_d = nc.dram_tensor("Wo", [C, C], dtm, kind="ExternalInput")
    sel_d = nc.dram_tensor("sel", [C, C], dtm, kind="ExternalInput")
    tri4_d = nc.dram_tensor("tri4", [128, 4 * 128], bf16, kind="ExternalInput")
    if bias_qk:
        bq_d = nc.dram_tensor("bq", [C, 1], f32, kind="ExternalInput")
        bk_d = nc.dram_tensor("bk", [C, 1], f32, kind="ExternalInput")
    if bias_v:
        bvb_d = nc.dram_tensor("bvb", [128, C], f32, kind="ExternalInput")
    if bias_o:
        bo_d = nc.dram_tensor("bo", [C, 1], f32, kind="ExternalInput")
    outT_d = nc.dram_tensor("outT", [C, T], f32, kind="ExternalOutput")

    sched = exp_engine_pattern(expmix, NPAIR)

    with tile.TileContext(nc) as tc:
        with contextlib.ExitStack() as ctx:
            ctx.enter_context(
                nc.allow_low_precision(reason="mixed precision attention")
            )
            consts = ctx.enter_context(tc.tile_pool(name="consts", bufs=1))
            work = ctx.enter_context(tc.tile_pool(name="work", bufs=2))
            wk2 = ctx.enter_context(tc.tile_pool(name="wk2", bufs=2))
            psS = ctx.enter_context(tc.tile_pool(name="psS", bufs=3, space="PSUM"))
            psM = ctx.enter_context(tc.tile_pool(name="psM", bufs=1, space="PSUM"))

            # persistent PSUM accumulators: the sums bank's non-head rows are
            # zeroed once here and never touched again, so a plain full-bank
            # copy (engines cannot stride the partition dim) reads zeros there
            pvacc = psM.tile([128, CW], f32, tag="pv")
            sumacc = psM.tile([128, CW], f32, tag="sm")
            nc.vector.memset(sumacc[:], 0.0)

            # ---- constants (outside the timing loop) ----
            xT = consts.tile([C, T], dtm)
            Wq = consts.tile([C, C], dtm)
            Wk = consts.tile([C, C], dtm)
            Wv = consts.tile([C, C], dtm)
            Wo = consts.tile([C, C], dtm)
            sel = consts.tile([C, C], dtm)
            tri4 = consts.tile([128, 4 * 128], bf16)
            nc.sync.dma_start(out=xT[:], in_=xT_d[:])
            nc.sync.dma_start(out=Wq[:], in_=Wq_d[:])
            nc.sync.dma_start(out=Wk[:], in_=Wk_d[:])
            nc.sync.dma_start(out=Wv[:], in_=Wv_d[:])
            nc.sync.dma_start(out=Wo[:], in_=Wo_d[:])
            nc.sync.dma_start(out=sel[:], in_=sel_d[:])
            nc.sync.dma_start(out=tri4[:], in_=tri4_d[:])
            if bias_qk:
                bq = consts.tile([C, 1], f32)
                bk = consts.tile([C, 1], f32)
                nc.sync.dma_start(out=bq[:], in_=bq_d[:])
                nc.sync.dma_start(out=bk[:], in_=bk_d[:])
            if bias_v:
                bvb = consts.tile([128, C], f32)
                nc.sync.dma_start(out=bvb[:], in_=bvb_d[:])
            if bias_o:
                bo = consts.tile([C, 1], f32)
                nc.sync.dma_start(out=bo[:], in_=bo_d[:])
            onescol = consts.tile([128, 1], bf16)
            # bf16 memset is rejected by codegen; fill via x*0 + 1
            nc.vector.tensor_scalar(onescol[:], tri4[:, 0:1], 0.0, 1.0,
                                    MUL, ADD)
            # sums landing pad: only rows {0,32,64,96} are ever written, the
            # rest must be zero for the selector matmul
            S1sb = consts.tile([128, NCH * CW], dtm)
            nc.vector.memset(S1sb[:].bitcast(mybir.dt.uint32), 0)

            tri4v = tri4[:].rearrange("p (h q) -> p h q", q=128)

            def body():
                qkT = work.tile([C, 2 * T], dtm, tag="qkT")
                kT = qkT[:, 0:T]
                qT = qkT[:, T:2 * T]
                v1 = work.tile([128, NBLK * H * Dh], bf16, tag="v1")
                v1v = v1.rearrange("p (j h e) -> p j h e", h=H, e=Dh)
                PBUF = work.tile([128, PRING * H * CW], bf16, tag="PBUF")

                units = [(c, j) for c in range(NCH)
                         for j in range(4 * (c + 1))]

                def emit_qkv(c):
                    cs = slice(c * CW, (c + 1) * CW)
                    qk = psS.tile([128, 1024], f32, tag="ps")
                    nc.tensor.matmul(qk[:, 0:CW], Wk[:], xT[:, cs],
                                     start=True, stop=True)
                    nc.tensor.matmul(qk[:, CW:2 * CW], Wq[:], xT[:, cs],
                                     start=True, stop=True)
                    if bias_qk:
                        nc.scalar.add(kT[:, cs], qk[:, 0:CW], bk)
                        nc.scalar.add(qT[:, cs], qk[:, CW:2 * CW], bq)
                    else:
                        dst = qkT[:].rearrange("p (g t) -> p g t", g=2)[
                            :, :, c * CW:(c + 1) * CW]
                        srcv = qk[:].rearrange("p (g q) -> p g q", g=2)
                        nc.scalar.copy(dst, srcv)
                    vs = psS.tile([128, 1024], f32, tag="ps")
                    for mj in range(4):
                        j = 4 * c + mj
                        nc.tensor.matmul(
                            vs[:, 128 * mj:128 * (mj + 1)],
                            xT[:, j * 128:(j + 1) * 128], Wv[:],
                            start=True, stop=True,
                        )
                    vsv = vs[:, 0:512].rearrange("p (j h e) -> p j h e",
                                                 h=H, e=Dh)
                    if bias_v:
                        bvbv = bvb[:].rearrange("p (h e) -> p h e", e=Dh)
                        for mj in range(4):
                            nc.vector.tensor_add(
                                v1v[:, 4 * c + mj, :, :], vsv[:, mj, :, :],
                                bvbv,
                            )
                    else:
                        nc.scalar.copy(v1v[:, 4 * c:4 * c + 4, :, :], vsv)

                def emit_scores_exp(idx, c, j):
                    m = j - 4 * c
                    lo = 128 * m if m > 0 else 0
                    P = PBUF[:, (idx % PRING) * H * CW:
                             ((idx % PRING) + 1) * H * CW]
                    Pv4 = P.rearrange("p (h q) -> p h q", q=CW)
                    for pi, (he, ho) in enumerate(((0, 1), (2, 3))):
                        sc = psS.tile([128, 1024], f32, tag="ps")
                        for col, h in ((0, he), (CW, ho)):
                            nc.tensor.matmul(
                                sc[:, col + lo:col + CW],
                                kT[32 * h:32 * h + 32,
                                   j * 128:(j + 1) * 128],
                                qT[32 * h:32 * h + 32,
                                   c * CW + lo:(c + 1) * CW],
                                start=True, stop=True,
                                tile_position=(32 * h, 0),
                            )
                        eng = sched[2 * idx + pi]
                        if lo:
                            pin = sc[:].rearrange(
                                "p (g q) -> p g q", q=CW)[:, :, lo:CW]
                            pout = Pv4[:, 2 * pi:2 * pi + 2, lo:CW]
                        else:
                            pin = sc[:]
                            pout = P[:, 1024 * pi:1024 * (pi + 1)]
                        if eng == "A":
                            nc.scalar.activation(pout, pin, EXP, scale=SCALE)
                        else:
                            nc.vector.tensor_scalar(
                                pout.bitcast(i16), pin, A16, B16, MUL, ADD)
                    if m >= 0:  # diagonal block: causal triangle mask
                        mv = Pv4[:, :, 128 * m:128 * (m + 1)]
                        nc.gpsimd.tensor_mul(mv, mv, tri4v)

                def emit_pv(idx, c, j):
                    m = j - 4 * c
                    lo = 128 * m if m > 0 else 0
                    nblk = 4 * (c + 1)
                    P = PBUF[:, (idx % PRING) * H * CW:
                             ((idx % PRING) + 1) * H * CW]
                    Pv4 = P.rearrange("p (h q) -> p h q", q=CW)
                    for h in range(H):
                        nc.tensor.matmul(
                            pvacc[32 * h:32 * h + 32, lo:CW],
                            v1v[:, j, h, :],
                            Pv4[:, h, lo:CW],
                            start=(j == 0), stop=(j == nblk - 1),
                            tile_position=(0, 32 * h),
                            skip_group_check=True,
                        )
                    for h in range(H):
                        nc.tensor.matmul(
                            sumacc[32 * h:32 * h + 1, lo:CW],
                            onescol[:],
                            Pv4[:, h, lo:CW],
                            start=(j == 0), stop=(j == nblk - 1),
                            tile_position=(0, 32 * h),
                            skip_group_check=True,
                        )

                def emit_chunk_end(c):
                    # reuse the freed accumulator banks: R broadcast lands in
                    # sumacc (after S1 copy), the projection in pvacc (after
                    # the OTs read) — no scores slot is stolen at chunk end
                    cs = slice(c * CW, (c + 1) * CW)
                    S1c = S1sb[:, c * CW:(c + 1) * CW]
                    nc.vector.tensor_copy(S1c, sumacc[:])
                    nc.tensor.matmul(sumacc[:], sel[:], S1c,
                                     start=True, stop=True)
                    Rsb = wk2.tile([128, CW], f32, tag="R", name=f"R{c}")
                    nc.vector.reciprocal(Rsb[:], sumacc[:])
                    OTs = wk2.tile([128, CW], dtm, tag="OTs", name=f"OTs{c}")
                    nc.vector.tensor_mul(OTs[:], pvacc[:], Rsb[:])
                    nc.tensor.matmul(pvacc[:], Wo[:], OTs[:],
                                     start=True, stop=True)
                    outsb = wk2.tile([128, CW], f32, tag="ob", name=f"ob{c}")
                    if bias_o:
                        nc.scalar.add(outsb[:], pvacc[:], bo)
                    else:
                        nc.scalar.copy(outsb[:], pvacc[:])
                    nc.sync.dma_start(out=outT_d[:, cs], in_=outsb[:])

                def emit_pv_and_end(idx):
                    c, j = units[idx]
                    emit_pv(idx, c, j)
                    if j == 4 * (c + 1) - 1:   # last s-block of chunk c
                        emit_chunk_end(c)

                for idx, (c, j) in enumerate(units):
                    if j == 0:
                        emit_qkv(c)
                    emit_scores_exp(idx, c, j)
                    if idx >= PVLAG:
                        emit_pv_and_end(idx - PVLAG)
                for idx in range(len(units) - PVLAG, len(units)):
                    emit_pv_and_end(idx)

            if iters == 1:
                body()
            else:
                from concourse import mybir as _mb

                with tc.For_i(0, iters, 1, hint_engines=(_mb.EngineType.PE,)):
                    body()

    return _split_excess_waits(nc) if fixup else nc


def _host_inputs(x, W_qkv, b_qkv, W_out, b_out):
    import ml_dtypes

    f = np.float32
    b16 = ml_dtypes.bfloat16
    Wq = np.ascontiguousarray(W_qkv[:, 0:C]).astype(b16)
    Wk = np.ascontiguousarray(W_qkv[:, C:2 * C]).astype(b16)
    Wv = np.ascontiguousarray(W_qkv[:, 2 * C:3 * C]).astype(b16)
    Wo = np.ascontiguousarray(W_out).astype(b16)
    sel = np.zeros((C, C), f)
    for g in range(H):
        sel[32 * g, 32 * g:32 * g + 32] = 1.0
    sel = sel.astype(b16)
    tri4 = np.tile(np.triu(np.ones((128, 128), f)), (1, 4)).astype(b16)
    bias_qk = bool(np.any(b_qkv[0:2 * C]))
    bias_v = bool(np.any(b_qkv[2 * C:3 * C]))
    bias_o = bool(np.any(b_out))
    base = {"Wq": Wq, "Wk": Wk, "Wv": Wv, "Wo": Wo, "sel": sel, "tri4": tri4}
    if bias_qk:
        base["bq"] = np.ascontiguousarray(b_qkv[0:C].reshape(C, 1), f)
        base["bk"] = np.ascontiguousarray(b_qkv[C:2 * C].reshape(C, 1), f)
    if bias_v:
        base["bvb"] = np.tile(b_qkv[2 * C:3 * C].reshape(1, C),
                              (128, 1)).astype(f)
    if bias_o:
        base["bo"] = np.ascontiguousarray(b_out.reshape(C, 1), f)
    in_maps = []
    for b in range(B):
        m = dict(base)
        m["xT"] = np.ascontiguousarray(x[b].T).astype(b16)
        in_maps.append(m)
    return in_maps, bias_qk, bias_v, bias_o


def kernel(x, W_qkv, b_qkv, W_out, b_out):
    from concourse.bass_utils import run_bass_kernel_spmd

    in_maps, bias_qk, bias_v, bias_o = _host_inputs(x, W_qkv, b_qkv, W_out,
                                                    b_out)
    nc = build_nc(mm="f32r", iters=1,
                  bias_qk=bias_qk, bias_v=bias_v, bias_o=bias_o)
    res = run_bass_kernel_spmd(nc, in_maps, core_ids=list(range(B)))
    out = np.stack([res.results[b]["outT"].T for b in range(B)])
    return np.ascontiguousarray(out, np.float32)
